# revision 1
# baseline (speedup 1.0000x reference)
"""AttentionPairBias Trainium2 Bass kernel.

Shapes (hardcoded): B=1, N=1024, C=768, CZ=128, H=16, D=48.
Sharding: query rows i split across 8 cores (128 rows each). Each core
reads its z row-block z[0, 128r:128r+128] (64MB), the full a, and the
weights; it produces output rows [128, 768]. Host gathers row blocks.

Math notes:
 - ln_a_w is folded into wq/wk/wv/wg on host; ln_a_b applied on device.
 - pair bias: w' = ln_z_w * w_z folded on host; bias = inv*(z-mu)@w'.
   The ln_z_b term (ln_z_b @ w_z) is constant over j -> softmax invariant
   -> dropped.
 - q scale 1/sqrt(D) folded into wq on host.
 - mask bias INF*(mask-1) is built on host as a [128,1024] broadcast and
   added to the assembled pair bias once per head slice.
"""

import math
import numpy as np
from contextlib import ExitStack

import concourse.bass as bass
import concourse.bacc as bacc
import concourse.mybir as mybir
import concourse.tile as tile
from concourse.bass_utils import run_bass_kernel_spmd

N, C, CZ, H, D = 1024, 768, 128, 16, 48
HD = H * D
NCORES = 8
RB = N // NCORES          # 128 rows per core
EPS = 1e-5
INF = 1e9
DP = 64                   # padded head dim (2 heads per 128 partitions)

F32 = mybir.dt.float32
F32R = mybir.dt.float32r
BF16 = mybir.dt.bfloat16
AX = mybir.AxisListType
AF = mybir.ActivationFunctionType
OP = mybir.AluOpType

# toggles
import os
PHASES = os.environ.get("KPHASES", "ABC")
Z_BF16 = True        # bf16 z pipeline after stats/normalize
ZT_CAST = False      # cast z->bf16 during the DMA itself (SWDGE)
PROBS_BF16 = False   # attention probs bf16 for transpose/o-matmul
BIAS_BF16 = True     # assembled pair-bias tensor dtype (SBUF budget needs it)


def _r(ap):
    """fp32 -> fp32r view for full-speed PE streaming."""
    return ap.bitcast(F32R) if ap.dtype == F32 else ap


def build_program():
    nc = bacc.Bacc("TRN2", target_bir_lowering=False, debug=False)

    def din(name, shape, dt=F32):
        return nc.dram_tensor(name, list(shape), dt,
                              kind="ExternalInput").ap()

    a_full = din("a_full", (N, C))
    a_own = din("a_own", (RB, C))
    z_blk = din("z_blk", (RB, N, CZ))
    wq_pad = din("wq_pad", (C, H * DP), F32R)
    wk_pad = din("wk_pad", (C, H * DP), F32R)
    wv_w = din("wv_w", (C, HD), F32R)
    wg_w = din("wg_w", (C, HD), F32R)
    wo_w = din("wo_w", (HD, C), F32R)
    wprime = nc.dram_tensor("wprime", [CZ, H], BF16,
                            kind="ExternalInput").ap()
    cq_row = din("cq_row", (1, H * DP), F32R)
    ck_row = din("ck_row", (1, H * DP), F32R)
    cv_row = din("cv_row", (1, HD), F32R)
    cg_row = din("cg_row", (1, HD), F32R)
    maskb_bc = din("maskb_bc", (128, N))
    bo_row = din("bo_row", (1, C), F32R)
    ones_col = din("ones_col", (1, 128), F32R)
    ones_row = din("ones_row", (1, 512), F32R)
    eye_f32 = din("eye_f32", (128, 128))
    eye_r = din("eye_r", (128, 128), F32R)
    eye_bf16 = nc.dram_tensor("eye_bf16", [128, 128], BF16,
                              kind="ExternalInput").ap()
    out_blk = nc.dram_tensor("out_blk", [RB, C], F32,
                             kind="ExternalOutput").ap()

    zdt = BF16 if Z_BF16 else F32
    pdt = BF16 if PROBS_BF16 else F32R
    bdt = BF16 if BIAS_BF16 else F32

    with tile.TileContext(nc) as tc, ExitStack() as ctx:
        const = ctx.enter_context(tc.tile_pool(name="const", bufs=1))
        persist = ctx.enter_context(tc.tile_pool(name="persist", bufs=1))
        pp_mm = ctx.enter_context(tc.tile_pool(name="pp_mm", bufs=3, space="PSUM"))
        pp_t = ctx.enter_context(tc.tile_pool(name="pp_t", bufs=2, space="PSUM"))
        pp_p = ctx.enter_context(tc.tile_pool(name="pp_p", bufs=1, space="PSUM"))
        pp_o = ctx.enter_context(tc.tile_pool(name="pp_o", bufs=1, space="PSUM"))

        # ---- constants to SBUF ----
        ident = const.tile([128, 128], F32)
        nc.sync.dma_start(ident[:], eye_f32)
        ident_b = const.tile([128, 128], BF16)
        nc.sync.dma_start(ident_b[:], eye_bf16)
        ident_r = const.tile([128, 128], F32R)
        nc.sync.dma_start(ident_r[:], eye_r)
        wp_sb = const.tile([CZ, H], BF16)
        nc.sync.dma_start(wp_sb[:], wprime)
        cq_sb = const.tile([1, H * DP], F32R)
        nc.sync.dma_start(cq_sb[:], cq_row)
        ck_sb = const.tile([1, H * DP], F32R)
        nc.sync.dma_start(ck_sb[:], ck_row)
        cv_sb = const.tile([1, HD], F32R)
        nc.sync.dma_start(cv_sb[:], cv_row)
        cg_sb = const.tile([1, HD], F32R)
        nc.sync.dma_start(cg_sb[:], cg_row)
        onesr_sb = const.tile([1, 512], F32R)
        nc.sync.dma_start(onesr_sb[:], ones_row)
        maskb_sb = const.tile([128, N], F32)
        nc.sync.dma_start(maskb_sb[:], maskb_bc)
        bo_sb = const.tile([1, C], F32R)
        nc.sync.dma_start(bo_sb[:], bo_row)
        ones_sb = const.tile([1, 128], F32R)
        nc.sync.dma_start(ones_sb[:], ones_col)
        eps_t = const.tile([128, 1], F32)
        nc.vector.memset(eps_t[:], EPS)

        # persistent activations
        kT_sb = persist.tile([128, 8, N], F32R, tag="kT")       # [2-head blk, b, j]
        v_sb = persist.tile([128, 8, HD], F32R, tag="v")        # [jp, jb, hd]
        qT_sb = persist.tile([128, 8, RB], F32R, tag="qT")      # [2-head blk, b, i]
        g_sb = persist.tile([128, HD], F32, tag="g")           # [i, hd]
        bias_sb = persist.tile([128, H, N], bdt, tag="bias")   # [i, h, j]
        rinv_sb = persist.tile([128, H], F32, tag="rinv")      # softmax 1/sum

        NT = N // 128  # 8 row tiles
        CC = C // 128  # 6 contraction chunks

        def layer_norm_tiles(pool, src_ap, ntiles, name):
            """LN over C for [ntiles*128, C] DRAM input -> SBUF [128, nt, C]."""
            x = pool.tile([128, ntiles, C], F32, tag=f"{name}_x")
            nc.sync.dma_start(
                x[:], src_ap.rearrange("(t p) c -> p t c", p=128))
            xn = pool.tile([128, ntiles, C], F32, tag=f"{name}_n")
            for t in range(ntiles):
                st = pool.tile([128, 2, 6], F32, tag=f"{name}_st")
                for g in range(2):
                    nc.vector.bn_stats(
                        st[:, g, :], x[:, t, g * 384:(g + 1) * 384])
                ag = pool.tile([128, 2], F32, tag=f"{name}_ag")
                nc.vector.bn_aggr(ag[:], st[:])
                sd = pool.tile([128, 1], F32, tag=f"{name}_sd")
                nc.scalar.activation(sd[:], ag[:, 1:2], AF.Sqrt, bias=eps_t[:])
                inv = pool.tile([128, 1], F32, tag=f"{name}_inv")
                nc.vector.reciprocal(inv[:], sd[:])
                nc.vector.tensor_scalar(
                    xn[:, t, :], x[:, t, :], ag[:, 0:1], inv[:],
                    op0=OP.subtract, op1=OP.mult)
            return xn

        def transpose_to(pool, src2d, nblk, name, dt=F32):
            """[128, nblk*128] SBUF -> [128, nblk, 128] transposed SBUF."""
            out = pool.tile([128, nblk, 128], dt, tag=f"{name}_T")
            sdt = src2d.dtype
            idn = {F32: ident[:], BF16: ident_b[:],
                   F32R: ident_r[:]}[sdt]
            b = 0
            while b < nblk:
                take = min(4, nblk - b)
                pt = pp_t.tile([128, 512], sdt, tag="tr")
                for k in range(take):
                    nc.tensor.transpose(
                        pt[:, k * 128:(k + 1) * 128],
                        src2d[:, (b + k) * 128:(b + k + 1) * 128], idn)
                nc.scalar.copy(
                    out[:, b:b + take, :],
                    pt[:, :take * 128].rearrange("p (t x) -> p t x", t=take))
                b += take
            return out

        # =========== phase A: a-side projections ===========
        with tc.tile_pool(name="aph", bufs=1) as aph:

            with tc.tile_pool(name="lnp", bufs=1) as lnp:
                a_n = layer_norm_tiles(lnp, a_full, NT, "af")     # [128, 8, C]
                # a_nT [c-part, cc, n]
                a_nT = aph.tile([128, CC, N], F32R, tag="anT")
                for t in range(NT):
                    pt6 = [pp_t.tile([128, 512], F32, tag="tr", name=f"pt6_{t}_{h2}")
                           for h2 in range(2)]
                    for cc in range(CC):
                        nc.tensor.transpose(
                            pt6[cc // 3][:, (cc % 3) * 128:(cc % 3 + 1) * 128],
                            a_n[:, t, cc * 128:(cc + 1) * 128], ident[:])
                    for half in range(2):
                        nc.scalar.copy(
                            a_nT[:, half * 3:(half + 1) * 3, t * 128:(t + 1) * 128],
                            pt6[half][:, :384].rearrange("p (t x) -> p t x", t=3))

            with tc.tile_pool(name="wkp", bufs=1) as wkp:
                wk_sb = wkp.tile([128, CC, H * DP], F32R, tag="wk")
                nc.sync.dma_start(wk_sb[:],
                                  wk_pad.rearrange("(t p) m -> p t m", p=128))
                # kT_pad [hd-pad block, j]: block b rows = heads 2b,2b+1
                for b in range(NT):
                    for jc in range(2):
                        ps = pp_mm.tile([128, 512], F32, tag="mm", name=f"k_{b}_{jc}")
                        for cc in range(CC):
                            nc.tensor.matmul(
                                ps[:], _r(wk_sb[:, cc, b * 128:(b + 1) * 128]),
                                _r(a_nT[:, cc, jc * 512:(jc + 1) * 512]),
                                start=(cc == 0), stop=(cc == CC - 1))
                        nc.scalar.copy(kT_sb[:, b, jc * 512:(jc + 1) * 512], ps[:])

            with tc.tile_pool(name="wvp", bufs=1) as wvp:
                wv_sb = wvp.tile([128, CC, HD], F32R, tag="wv")
                nc.sync.dma_start(wv_sb[:],
                                  wv_w.rearrange("(t p) m -> p t m", p=128))
                for t in range(NT):
                    for fc in range(2):
                        ps = pp_mm.tile([128, 384], F32, tag="mm", name=f"v_{t}_{fc}")
                        for cc in range(CC):
                            nc.tensor.matmul(
                                ps[:], _r(a_nT[:, cc, t * 128:(t + 1) * 128]),
                                _r(wv_sb[:, cc, fc * 384:(fc + 1) * 384]),
                                start=(cc == 0), stop=(cc == CC - 1))
                        nc.scalar.copy(v_sb[:, t, fc * 384:(fc + 1) * 384], ps[:])

            # own-row LN + q/g
            ao_n = layer_norm_tiles(aph, a_own, 1, "ao")          # [128, 1, C]
            ao_T = transpose_to(aph, ao_n[:, 0, :], CC, "aoT", dt=F32R)    # [c, cc, 128]

            with tc.tile_pool(name="wqp", bufs=1) as wqp:
                wq_sb = wqp.tile([128, CC, H * DP], F32R, tag="wq")
                nc.sync.dma_start(wq_sb[:],
                                  wq_pad.rearrange("(t p) m -> p t m", p=128))
                qtmp = wqp.tile([128, H * DP], F32R, tag="qtmp")
                for fc in range(2):
                    ps = pp_mm.tile([128, 512], F32, tag="mm", name=f"q_{fc}")
                    for cc in range(CC):
                        nc.tensor.matmul(
                            ps[:], _r(ao_T[:, cc, :]),
                            _r(wq_sb[:, cc, fc * 512:(fc + 1) * 512]),
                            start=(cc == 0), stop=(cc == CC - 1))
                    nc.scalar.copy(qtmp[:, fc * 512:(fc + 1) * 512], ps[:])
                qT_t = transpose_to(wqp, qtmp[:], NT, "qT", dt=F32R)
                nc.vector.tensor_copy(qT_sb[:], qT_t[:])

            with tc.tile_pool(name="wgp", bufs=1) as wgp:
                wg_sb = wgp.tile([128, CC, HD], F32R, tag="wg")
                nc.sync.dma_start(wg_sb[:],
                                  wg_w.rearrange("(t p) m -> p t m", p=128))
                # g = sigmoid(ao_n @ wg + bg); bg via rank-1 matmul
                for fc in range(2):
                    ps = pp_mm.tile([128, 384], F32, tag="mm", name=f"g_{fc}")
                    for cc in range(CC):
                        nc.tensor.matmul(
                            ps[:], _r(ao_T[:, cc, :]),
                            _r(wg_sb[:, cc, fc * 384:(fc + 1) * 384]),
                            start=(cc == 0), stop=False)
                    nc.tensor.matmul(
                        ps[:], _r(ones_sb[:]),
                        cg_sb[:, fc * 384:(fc + 1) * 384],
                        start=False, stop=True)
                    nc.scalar.activation(g_sb[:, fc * 384:(fc + 1) * 384], ps[:],
                                         AF.Sigmoid)

        # =========== phase B: z pipeline ===========
        if "B" not in PHASES:
            zrng = []
        else:
            zrng = list(range(RB))
        zpool = ctx.enter_context(tc.tile_pool(name="zpool", bufs=2))
        stpool = ctx.enter_context(tc.tile_pool(name="stage", bufs=1))
        dpool = ctx.enter_context(tc.tile_pool(name="dstage", bufs=2,
                                               space="DRAM"))

        for i in zrng:
            zt = zpool.tile([128, NT, CZ], zdt if ZT_CAST else F32,
                            tag="zt")
            if ZT_CAST:
                nc.gpsimd.dma_start(
                    zt[:], z_blk[i].rearrange("(jb jp) c -> jp jb c", jp=128))
            else:
                nc.sync.dma_start(
                    zt[:], z_blk[i].rearrange("(jb jp) c -> jp jb c", jp=128))
            # stats: one bn_stats via non-mergeable 4D view (group g'=b*2+a
            # holds stats of jb=a*4+b), then per-group bn_aggr.
            st = zpool.tile([128, NT, 8], F32, tag="zst")
            ag = zpool.tile([128, NT, 2], F32, tag="zag")
            for jb in range(NT):
                nc.vector.bn_stats(st[:, jb, 0:6], zt[:, jb, :])
                nc.vector.bn_aggr(ag[:, jb, :], st[:, jb, 0:6])
            sd = zpool.tile([128, NT], F32, tag="zsd")
            nc.scalar.activation(sd[:], ag[:, :, 1], AF.Sqrt, bias=eps_t[:])
            inv = zpool.tile([128, NT], F32, tag="zinv")
            nc.vector.reciprocal(inv[:], sd[:])
            mi = zpool.tile([128, NT], F32, tag="zmi")
            nc.vector.tensor_mul(mi[:], ag[:, :, 0], inv[:])
            nmi = zpool.tile([128, NT], F32, tag="znmi")
            nc.vector.tensor_scalar_mul(nmi[:], mi[:], -1.0)
            zn = zpool.tile([128, NT, CZ], zdt, tag="zn")
            for jb in range(NT):
                nc.gpsimd.tensor_scalar(
                    zn[:, jb, :], zt[:, jb, :], inv[:, jb:jb + 1],
                    nmi[:, jb:jb + 1], op0=OP.mult, op1=OP.add)
            zT = transpose_to(zpool, zn[:].rearrange("p t c -> p (t c)"),
                              NT, "zT", dt=zdt)
            pps = pp_p.tile([16, N], F32, tag="pp", name=f"pps_{i}")
            for half in range(2):
                nc.tensor.matmul(
                    pps[:, half * 512:(half + 1) * 512],
                    _r(wp_sb[:]),
                    _r(zT[:, half * 4:(half + 1) * 4, :]
                       .rearrange("p t x -> p (t x)")),
                    start=True, stop=True)
            if i % 8 == 0:
                pstg = stpool.tile([16, 8, N], bdt, tag="pstg",
                                   name=f"pstg_{i}")
            nc.scalar.copy(pstg[:, i % 8, :], pps[:])
            if i % 8 == 7:
                pb = dpool.tile([16, 8, N], bdt, tag="pb", name=f"pb_{i}")
                nc.sync.dma_start(pb[:], pstg[:])
                nc.sync.dma_start(
                    bias_sb[i - 7:i + 1, :, :],
                    pb[:].rearrange("h i j -> i h j"))

        # mask bias add (per h) — bias_sb += maskb
        for h in (range(H) if "B" in PHASES else []):
            nc.vector.tensor_add(bias_sb[:, h, :], bias_sb[:, h, :],
                                 maskb_sb[:])

        # =========== phase C: attention per head ===========
        hpool = ctx.enter_context(tc.tile_pool(name="hpool", bufs=2))
        o_sb = persist.tile([128, HD], F32, tag="o")
        CSUB = 3 if "C" in PHASES else 0
        if "C1" in PHASES:
            CSUB = 1
        elif "C2" in PHASES:
            CSUB = 2
        if CSUB < 3:
            nc.vector.memset(o_sb[:], 1.0)
            nc.vector.memset(rinv_sb[:], 1.0)
        import os as _os
        _hsel = _os.environ.get("KHEADS", "all")
        _heads = (range(H) if _hsel == "all"
                  else range(0, H, 2) if _hsel == "even" else [])
        for h in (_heads if CSUB >= 1 else []):
            b, m = h // 2, h % 2
            l_sb = hpool.tile([128, N], F32, tag="l")
            mx = hpool.tile([128, 2], F32, tag="mx")
            for jc in range(2):
                ps = pp_mm.tile([128, 512], F32, tag="mm", name=f"qk_{h}_{jc}")
                nc.tensor.matmul(
                    ps[:],
                    _r(qT_sb[m * DP:m * DP + D, b, :]),
                    _r(kT_sb[m * DP:m * DP + D, b, jc * 512:(jc + 1) * 512]),
                    start=True, stop=True)
                nc.vector.tensor_add(
                    l_sb[:, jc * 512:(jc + 1) * 512], ps[:],
                    bias_sb[:, h, jc * 512:(jc + 1) * 512])
            nc.vector.tensor_reduce(
                mx[:, 1:2], l_sb[:], op=OP.max, axis=AX.X)
            if CSUB < 2:
                continue
            nm = hpool.tile([128, 1], F32, tag="nm")
            nc.vector.tensor_scalar_mul(nm[:], mx[:, 1:2], -1.0)
            probs = hpool.tile([128, N], pdt, tag="probs")
            ssum = hpool.tile([128, 1], F32, tag="ssum")
            nc.scalar.activation(probs[:], l_sb[:], AF.Exp, bias=nm[:],
                                 accum_out=ssum[:])
            nc.vector.reciprocal(rinv_sb[:, h:h + 1], ssum[:])
            if CSUB < 3:
                continue
            pT = transpose_to(hpool, probs[:], NT, "pT", dt=pdt)
            po = pp_o.tile([128, D], F32, tag="oo", name=f"po_{h}")
            for jb in range(NT):
                nc.tensor.matmul(
                    po[:], _r(pT[:, jb, :]),
                    _r(v_sb[:, jb, h * D:(h + 1) * D]),
                    start=(jb == 0), stop=(jb == NT - 1))
            nc.scalar.activation(o_sb[:, h * D:(h + 1) * D], po[:], AF.Copy,
                                 scale=rinv_sb[:, h:h + 1])
        if CSUB in (1, 2):
            nc.vector.memset(o_sb[:, 0:1], 0.0)
            nc.vector.tensor_scalar(o_sb[:, 0:1], l_sb[:, 0:1], 1e-6, None,
                                    op0=OP.mult, op1=OP.bypass)

        # =========== phase D: output projection ===========
        opool = ctx.enter_context(tc.tile_pool(name="opool", bufs=1))
        wo_sb = opool.tile([128, CC, C], F32R, tag="wo")
        nc.sync.dma_start(wo_sb[:], wo_w.rearrange("(t p) m -> p t m", p=128))
        og = opool.tile([128, HD], F32R, tag="og")
        nc.vector.tensor_mul(og[:], o_sb[:], g_sb[:])
        ogT = transpose_to(opool, og[:], CC, "ogT", dt=F32R)
        out_s = opool.tile([128, C], F32, tag="outs")
        for fc in range(2):
            ps = pp_mm.tile([128, 384], F32, tag="mm")
            for cc in range(CC):
                nc.tensor.matmul(
                    ps[:], _r(ogT[:, cc, :]),
                    _r(wo_sb[:, cc, fc * 384:(fc + 1) * 384]),
                    start=(cc == 0), stop=False)
            nc.tensor.matmul(
                ps[:], _r(ones_sb[:]), _r(bo_sb[:, fc * 384:(fc + 1) * 384]),
                start=False, stop=True)
            nc.scalar.copy(out_s[:, fc * 384:(fc + 1) * 384], ps[:])
        nc.sync.dma_start(out_blk, out_s[:])

    nc.compile()
    return nc


def _host_prep(a, z, mask, ln_a_w, ln_a_b, ln_z_w, ln_z_b, w_z,
               wq, wk, wv, wg, bg, wo, bo):
    f = np.float32
    wq_f = (ln_a_w[:, None] * wq).astype(f) / math.sqrt(D)
    wk_f = (ln_a_w[:, None] * wk).astype(f)
    wq_pad = np.zeros((C, H * DP), f)
    wk_pad = np.zeros((C, H * DP), f)
    for h in range(H):
        wq_pad[:, h * DP:h * DP + D] = wq_f[:, h * D:(h + 1) * D]
        wk_pad[:, h * DP:h * DP + D] = wk_f[:, h * D:(h + 1) * D]
    wq_orig_pad = np.zeros((C, H * DP), f)
    wk_orig_pad = np.zeros((C, H * DP), f)
    for h in range(H):
        wq_orig_pad[:, h * DP:h * DP + D] = wq[:, h * D:(h + 1) * D] / math.sqrt(D)
        wk_orig_pad[:, h * DP:h * DP + D] = wk[:, h * D:(h + 1) * D]
    shared = {
        "a_full": np.ascontiguousarray(a[0]).astype(f),
        "cq_row": (ln_a_b @ wq_orig_pad).reshape(1, H * DP).astype(f),
        "ck_row": (ln_a_b @ wk_orig_pad).reshape(1, H * DP).astype(f),
        "cv_row": (ln_a_b @ wv).reshape(1, HD).astype(f),
        "cg_row": (ln_a_b @ wg + bg).reshape(1, HD).astype(f),
        "wq_pad": wq_pad,
        "wk_pad": wk_pad,
        "wv_w": (ln_a_w[:, None] * wv).astype(f),
        "wg_w": (ln_a_w[:, None] * wg).astype(f),
        "wo_w": wo.astype(f),
        "wprime": (ln_z_w[:, None] * w_z).astype(f),  # cast below
        "maskb_bc": np.broadcast_to(INF * (mask[0] - 1.0), (128, N))
        .astype(f).copy(),
        "bo_row": bo.reshape(1, C).astype(f),
        "ones_col": np.ones((1, 128), f),
        "ones_row": np.ones((1, 512), f),
        "eye_f32": np.eye(128, dtype=f),
        "eye_r": np.eye(128, dtype=f),
        "eye_bf16": np.eye(128, dtype=f),  # cast below
    }
    import ml_dtypes
    shared["eye_bf16"] = shared["eye_bf16"].astype(ml_dtypes.bfloat16)
    shared["wprime"] = shared["wprime"].astype(ml_dtypes.bfloat16)
    return shared


def _run(inputs, trace=False, **spmd_kwargs):
    shared = _host_prep(**inputs)
    a, z = inputs["a"], inputs["z"]
    nc = build_program()
    in_maps = []
    for r in range(NCORES):
        m = dict(shared)
        m["a_own"] = np.ascontiguousarray(
            a[0, r * RB:(r + 1) * RB]).astype(np.float32)
        m["z_blk"] = np.ascontiguousarray(
            z[0, r * RB:(r + 1) * RB]).astype(np.float32)
        in_maps.append(m)
    res = run_bass_kernel_spmd(nc, in_maps, list(range(NCORES)),
                               trace=trace, **spmd_kwargs)
    out = np.concatenate([res.results[r]["out_blk"] for r in range(NCORES)],
                         axis=0)
    return out.reshape(1, N, C).astype(np.float32), res


def kernel(**inputs):
    out, _ = _run(inputs)
    return out


if __name__ == "__main__":
    nc = build_program()
    print("program built ok")



# revision 8
# speedup vs baseline: 66.4287x; 66.4287x over previous
"""AttentionPairBias Trainium2 Bass kernel.

Shapes (hardcoded): B=1, N=1024, C=768, CZ=128, H=16, D=48.
Sharding: query rows i split across 8 cores (128 rows each). Each core
reads its z row-block z[0, 128r:128r+128] (64MB), the full a, and the
weights; it produces output rows [128, 768]. Host gathers row blocks.

Math notes:
 - ln_a_w is folded into wq/wk/wv/wg on host; ln_a_b applied on device.
 - pair bias: w' = ln_z_w * w_z folded on host; bias = inv*(z-mu)@w'.
   The ln_z_b term (ln_z_b @ w_z) is constant over j -> softmax invariant
   -> dropped.
 - q scale 1/sqrt(D) folded into wq on host.
 - mask bias INF*(mask-1) is built on host as a [128,1024] broadcast and
   added to the assembled pair bias once per head slice.
"""

import math
import numpy as np
from contextlib import ExitStack

import concourse.bass as bass
import concourse.bacc as bacc
import concourse.mybir as mybir
import concourse.tile as tile
from concourse.bass_utils import run_bass_kernel_spmd

N, C, CZ, H, D = 1024, 768, 128, 16, 48
HD = H * D
NCORES = 8
RB = N // NCORES          # 128 rows per core
EPS = 1e-5
INF = 1e9
DP = 64                   # padded head dim (2 heads per 128 partitions)

F32 = mybir.dt.float32
F32R = mybir.dt.float32r
BF16 = mybir.dt.bfloat16
AX = mybir.AxisListType
AF = mybir.ActivationFunctionType
OP = mybir.AluOpType

# toggles
import os
PHASES = os.environ.get("KPHASES", "ABC")
KREPEAT = int(os.environ.get("KREPEAT", "1"))
Z_BF16 = True        # bf16 z pipeline after stats/normalize
ZT_CAST = False      # cast z->bf16 during the DMA itself (SWDGE)
PROBS_BF16 = False   # attention probs bf16 for transpose/o-matmul
BIAS_BF16 = True     # assembled pair-bias tensor dtype (SBUF budget needs it)


def _r(ap):
    """fp32 -> fp32r view for full-speed PE streaming."""
    return ap.bitcast(F32R) if ap.dtype == F32 else ap


def build_program():
    nc = bacc.Bacc("TRN2", target_bir_lowering=False, debug=False)

    def din(name, shape, dt=F32):
        return nc.dram_tensor(name, list(shape), dt,
                              kind="ExternalInput").ap()

    a_full = din("a_full", (N, C))
    a_own = din("a_own", (RB, C))
    z_blk = din("z_blk", (RB, N, CZ))
    wq_pad = din("wq_pad", (C, H * DP), F32R)
    wk_pad = din("wk_pad", (C, H * DP), F32R)
    wv_w = din("wv_w", (C, HD), F32R)
    wg_w = din("wg_w", (C, HD), F32R)
    wo_w = din("wo_w", (HD, C), F32R)
    wprime = nc.dram_tensor("wprime", [CZ, H], BF16,
                            kind="ExternalInput").ap()
    cq_row = din("cq_row", (1, H * DP), F32R)
    ck_row = din("ck_row", (1, H * DP), F32R)
    cv_row = din("cv_row", (1, HD), F32R)
    cg_row = din("cg_row", (1, HD), F32R)
    maskb_bc = din("maskb_bc", (128, N))
    bo_row = din("bo_row", (1, C), F32R)
    ones_col = din("ones_col", (1, 128), F32R)
    ones_row = din("ones_row", (1, 512), F32R)
    eye_f32 = din("eye_f32", (128, 128))
    eye_r = din("eye_r", (128, 128), F32R)
    eye_bf16 = nc.dram_tensor("eye_bf16", [128, 128], BF16,
                              kind="ExternalInput").ap()
    out_blk = nc.dram_tensor("out_blk", [RB, C], F32,
                             kind="ExternalOutput").ap()

    zdt = BF16 if Z_BF16 else F32
    pdt = BF16 if PROBS_BF16 else F32R
    bdt = BF16 if BIAS_BF16 else F32

    with tile.TileContext(nc) as tc:
     for _rep in range(KREPEAT):
      sfx = f"_{_rep}" if KREPEAT > 1 else ""
      with ExitStack() as ctx:
        const = ctx.enter_context(tc.tile_pool(name="const" + sfx, bufs=1))
        persist = ctx.enter_context(tc.tile_pool(name="persist", bufs=1))
        pp_mm = ctx.enter_context(tc.tile_pool(name="pp_mm", bufs=3, space="PSUM"))
        pp_t = ctx.enter_context(tc.tile_pool(name="pp_t", bufs=2, space="PSUM"))
        pp_p = ctx.enter_context(tc.tile_pool(name="pp_p", bufs=1, space="PSUM"))
        pp_o = ctx.enter_context(tc.tile_pool(name="pp_o", bufs=1, space="PSUM"))

        # ---- constants to SBUF ----
        ident = const.tile([128, 128], F32)
        nc.sync.dma_start(ident[:], eye_f32)
        ident_b = const.tile([128, 128], BF16)
        nc.sync.dma_start(ident_b[:], eye_bf16)
        ident_r = const.tile([128, 128], F32R)
        nc.sync.dma_start(ident_r[:], eye_r)
        wp_sb = const.tile([CZ, H], BF16)
        nc.sync.dma_start(wp_sb[:], wprime)
        cq_sb = const.tile([1, H * DP], F32R)
        nc.sync.dma_start(cq_sb[:], cq_row)
        ck_sb = const.tile([1, H * DP], F32R)
        nc.sync.dma_start(ck_sb[:], ck_row)
        cv_sb = const.tile([1, HD], F32R)
        nc.sync.dma_start(cv_sb[:], cv_row)
        cg_sb = const.tile([1, HD], F32R)
        nc.sync.dma_start(cg_sb[:], cg_row)
        onesr_sb = const.tile([1, 512], F32R)
        nc.sync.dma_start(onesr_sb[:], ones_row)
        maskb_sb = const.tile([128, N], F32)
        nc.sync.dma_start(maskb_sb[:], maskb_bc)
        bo_sb = const.tile([1, C], F32R)
        nc.sync.dma_start(bo_sb[:], bo_row)
        ones_sb = const.tile([1, 128], F32R)
        nc.sync.dma_start(ones_sb[:], ones_col)
        eps_t = const.tile([128, 1], F32)
        nc.vector.memset(eps_t[:], EPS)

        # persistent activations
        kT_sb = persist.tile([128, 8, N], F32R, tag="kT")       # [2-head blk, b, j]
        v_sb = persist.tile([128, 8, HD], F32R, tag="v")        # [jp, jb, hd]
        qT_sb = persist.tile([128, 8, RB], F32R, tag="qT")      # [2-head blk, b, i]
        g_sb = persist.tile([128, HD], F32, tag="g")           # [i, hd]
        bias_sb = persist.tile([128, H, N], bdt, tag="bias")   # [i, h, j]
        rinv_sb = persist.tile([128, H], F32, tag="rinv")      # softmax 1/sum

        NT = N // 128  # 8 row tiles
        CC = C // 128  # 6 contraction chunks

        def layer_norm_tiles(pool, src_ap, ntiles, name):
            """LN over C for [ntiles*128, C] DRAM input -> SBUF [128, nt, C]."""
            x = pool.tile([128, ntiles, C], F32, tag=f"{name}_x")
            nc.sync.dma_start(
                x[:], src_ap.rearrange("(t p) c -> p t c", p=128))
            xn = pool.tile([128, ntiles, C], F32, tag=f"{name}_n")
            for t in range(ntiles):
                st = pool.tile([128, 2, 6], F32, tag=f"{name}_st")
                for g in range(2):
                    nc.vector.bn_stats(
                        st[:, g, :], x[:, t, g * 384:(g + 1) * 384])
                ag = pool.tile([128, 2], F32, tag=f"{name}_ag")
                nc.vector.bn_aggr(ag[:], st[:])
                sd = pool.tile([128, 1], F32, tag=f"{name}_sd")
                nc.scalar.activation(sd[:], ag[:, 1:2], AF.Sqrt, bias=eps_t[:])
                inv = pool.tile([128, 1], F32, tag=f"{name}_inv")
                nc.vector.reciprocal(inv[:], sd[:])
                nc.vector.tensor_scalar(
                    xn[:, t, :], x[:, t, :], ag[:, 0:1], inv[:],
                    op0=OP.subtract, op1=OP.mult)
            return xn

        def transpose_to(pool, src2d, nblk, name, dt=F32):
            """[128, nblk*128] SBUF -> [128, nblk, 128] transposed SBUF."""
            out = pool.tile([128, nblk, 128], dt, tag=f"{name}_T")
            sdt = src2d.dtype
            idn = {F32: ident[:], BF16: ident_b[:],
                   F32R: ident_r[:]}[sdt]
            b = 0
            while b < nblk:
                take = min(4, nblk - b)
                pt = pp_t.tile([128, 512], sdt, tag="tr")
                for k in range(take):
                    nc.tensor.transpose(
                        pt[:, k * 128:(k + 1) * 128],
                        src2d[:, (b + k) * 128:(b + k + 1) * 128], idn)
                nc.scalar.copy(
                    out[:, b:b + take, :],
                    pt[:, :take * 128].rearrange("p (t x) -> p t x", t=take))
                b += take
            return out

        # =========== phase A: a-side projections ===========
        with tc.tile_pool(name="aph", bufs=1) as aph:

            with tc.tile_pool(name="lnp", bufs=1) as lnp:
                a_n = layer_norm_tiles(lnp, a_full, NT, "af")     # [128, 8, C]
                # a_nT [c-part, cc, n]
                a_nT = aph.tile([128, CC, N], F32R, tag="anT")
                for t in range(NT):
                    pt6 = [pp_t.tile([128, 512], F32, tag="tr", name=f"pt6_{t}_{h2}")
                           for h2 in range(2)]
                    for cc in range(CC):
                        nc.tensor.transpose(
                            pt6[cc // 3][:, (cc % 3) * 128:(cc % 3 + 1) * 128],
                            a_n[:, t, cc * 128:(cc + 1) * 128], ident[:])
                    for half in range(2):
                        nc.scalar.copy(
                            a_nT[:, half * 3:(half + 1) * 3, t * 128:(t + 1) * 128],
                            pt6[half][:, :384].rearrange("p (t x) -> p t x", t=3))

            with tc.tile_pool(name="wkp", bufs=1) as wkp:
                wk_sb = wkp.tile([128, CC, H * DP], F32R, tag="wk")
                nc.sync.dma_start(wk_sb[:],
                                  wk_pad.rearrange("(t p) m -> p t m", p=128))
                # kT_pad [hd-pad block, j]: block b rows = heads 2b,2b+1
                for b in range(NT):
                    for jc in range(2):
                        ps = pp_mm.tile([128, 512], F32, tag="mm", name=f"k_{b}_{jc}")
                        for cc in range(CC):
                            nc.tensor.matmul(
                                ps[:], _r(wk_sb[:, cc, b * 128:(b + 1) * 128]),
                                _r(a_nT[:, cc, jc * 512:(jc + 1) * 512]),
                                start=(cc == 0), stop=(cc == CC - 1))
                        nc.scalar.copy(kT_sb[:, b, jc * 512:(jc + 1) * 512], ps[:])

            with tc.tile_pool(name="wvp", bufs=1) as wvp:
                wv_sb = wvp.tile([128, CC, HD], F32R, tag="wv")
                nc.sync.dma_start(wv_sb[:],
                                  wv_w.rearrange("(t p) m -> p t m", p=128))
                for t in range(NT):
                    for fc in range(2):
                        ps = pp_mm.tile([128, 384], F32, tag="mm", name=f"v_{t}_{fc}")
                        for cc in range(CC):
                            nc.tensor.matmul(
                                ps[:], _r(a_nT[:, cc, t * 128:(t + 1) * 128]),
                                _r(wv_sb[:, cc, fc * 384:(fc + 1) * 384]),
                                start=(cc == 0), stop=(cc == CC - 1))
                        nc.scalar.copy(v_sb[:, t, fc * 384:(fc + 1) * 384], ps[:])

            # own-row LN + q/g
            ao_n = layer_norm_tiles(aph, a_own, 1, "ao")          # [128, 1, C]
            ao_T = transpose_to(aph, ao_n[:, 0, :], CC, "aoT", dt=F32R)    # [c, cc, 128]

            with tc.tile_pool(name="wqp", bufs=1) as wqp:
                wq_sb = wqp.tile([128, CC, H * DP], F32R, tag="wq")
                nc.sync.dma_start(wq_sb[:],
                                  wq_pad.rearrange("(t p) m -> p t m", p=128))
                qtmp = wqp.tile([128, H * DP], F32R, tag="qtmp")
                for fc in range(2):
                    ps = pp_mm.tile([128, 512], F32, tag="mm", name=f"q_{fc}")
                    for cc in range(CC):
                        nc.tensor.matmul(
                            ps[:], _r(ao_T[:, cc, :]),
                            _r(wq_sb[:, cc, fc * 512:(fc + 1) * 512]),
                            start=(cc == 0), stop=(cc == CC - 1))
                    nc.scalar.copy(qtmp[:, fc * 512:(fc + 1) * 512], ps[:])
                qT_t = transpose_to(wqp, qtmp[:], NT, "qT", dt=F32R)
                nc.vector.tensor_copy(qT_sb[:], qT_t[:])

            with tc.tile_pool(name="wgp", bufs=1) as wgp:
                wg_sb = wgp.tile([128, CC, HD], F32R, tag="wg")
                nc.sync.dma_start(wg_sb[:],
                                  wg_w.rearrange("(t p) m -> p t m", p=128))
                # g = sigmoid(ao_n @ wg + bg); bg via rank-1 matmul
                for fc in range(2):
                    ps = pp_mm.tile([128, 384], F32, tag="mm", name=f"g_{fc}")
                    for cc in range(CC):
                        nc.tensor.matmul(
                            ps[:], _r(ao_T[:, cc, :]),
                            _r(wg_sb[:, cc, fc * 384:(fc + 1) * 384]),
                            start=(cc == 0), stop=False)
                    nc.tensor.matmul(
                        ps[:], _r(ones_sb[:]),
                        cg_sb[:, fc * 384:(fc + 1) * 384],
                        start=False, stop=True)
                    nc.scalar.activation(g_sb[:, fc * 384:(fc + 1) * 384], ps[:],
                                         AF.Sigmoid)

        # =========== phase B: z pipeline ===========
        if "B" not in PHASES:
            zrng = []
        else:
            zrng = list(range(RB))
        zpool = ctx.enter_context(tc.tile_pool(name="zpool", bufs=2))
        stpool = ctx.enter_context(tc.tile_pool(name="stage", bufs=1))
        dpool = ctx.enter_context(tc.tile_pool(name="dstage", bufs=2,
                                               space="DRAM"))

        zstate = {}
        for i in zrng:
            # batch 2 i-rows per z DMA (amortizes the ~2us per-DMA
            # completion serialization on the sync queue)
            if i % 2 == 0:
                zt2 = zpool.tile([128, 2, NT, CZ], F32, tag="zt")
                nc.sync.dma_start(
                    zt2[:], z_blk[i:i + 2].rearrange(
                        "i (jb jp) c -> jp i jb c", jp=128))
                zstate["zt2"] = zt2
            zt = zstate["zt2"][:, i % 2]
            # stats: one bn_stats via non-mergeable 4D view (group g'=b*2+a
            # holds stats of jb=a*4+b), then per-group bn_aggr.
            st = zpool.tile([128, NT, 8], F32, tag="zst")
            ag = zpool.tile([128, NT, 2], F32, tag="zag")
            for jb in range(NT):
                nc.vector.bn_stats(st[:, jb, 0:6], zt[:, jb, :])
                nc.vector.bn_aggr(ag[:, jb, :], st[:, jb, 0:6])
            sd = zpool.tile([128, NT], F32, tag="zsd")
            nc.scalar.activation(sd[:], ag[:, :, 1], AF.Sqrt, bias=eps_t[:])
            inv = zpool.tile([128, NT], F32, tag="zinv")
            nc.vector.reciprocal(inv[:], sd[:])
            mi = zpool.tile([128, NT], F32, tag="zmi")
            nc.vector.tensor_mul(mi[:], ag[:, :, 0], inv[:])
            nmi = zpool.tile([128, NT], F32, tag="znmi")
            nc.vector.tensor_scalar_mul(nmi[:], mi[:], -1.0)
            zn = zpool.tile([128, NT, CZ], zdt, tag="zn")
            for jb in range(NT):
                nc.gpsimd.tensor_scalar(
                    zn[:, jb, :], zt[:, jb, :], inv[:, jb:jb + 1],
                    nmi[:, jb:jb + 1], op0=OP.mult, op1=OP.add)
            zT = transpose_to(zpool, zn[:].rearrange("p t c -> p (t c)"),
                              NT, "zT", dt=zdt)
            pps = pp_p.tile([16, N], F32, tag="pp", name=f"pps_{i}")
            for half in range(2):
                nc.tensor.matmul(
                    pps[:, half * 512:(half + 1) * 512],
                    _r(wp_sb[:]),
                    _r(zT[:, half * 4:(half + 1) * 4, :]
                       .rearrange("p t x -> p (t x)")),
                    start=True, stop=True)
            if i % 8 == 0:
                pstg = stpool.tile([16, 8, N], bdt, tag="pstg",
                                   name=f"pstg_{i}")
            nc.scalar.copy(pstg[:, i % 8, :], pps[:])
            if i % 8 == 7:
                pb = dpool.tile([16, 8, N], bdt, tag="pb", name=f"pb_{i}")
                # issue on the ACT HWDGE queue: keeps the compute-dependent
                # bias flip off the z-load queue (sync FIFO would serialize
                # the next z DMA behind this tile's whole compute chain)
                nc.scalar.dma_start(pb[:], pstg[:])
                nc.scalar.dma_start(
                    bias_sb[i - 7:i + 1, :, :],
                    pb[:].rearrange("h i j -> i h j"))

        # mask bias add (per h) — bias_sb += maskb
        for h in (range(H) if "B" in PHASES else []):
            nc.vector.tensor_add(bias_sb[:, h, :], bias_sb[:, h, :],
                                 maskb_sb[:])

        # =========== phase C: attention per head ===========
        hpool = ctx.enter_context(tc.tile_pool(name="hpool", bufs=2))
        o_sb = persist.tile([128, HD], F32, tag="o")
        CSUB = 3 if "C" in PHASES else 0
        if "C1" in PHASES:
            CSUB = 1
        elif "C2" in PHASES:
            CSUB = 2
        if CSUB < 3:
            nc.vector.memset(o_sb[:], 1.0)
            nc.vector.memset(rinv_sb[:], 1.0)
        import os as _os
        _hsel = _os.environ.get("KHEADS", "all")
        _heads = (range(H) if _hsel == "all"
                  else range(0, H, 2) if _hsel == "even" else [])
        for h in (_heads if CSUB >= 1 else []):
            b, m = h // 2, h % 2
            l_sb = hpool.tile([128, N], F32, tag="l")
            mx = hpool.tile([128, 2], F32, tag="mx")
            for jc in range(2):
                ps = pp_mm.tile([128, 512], F32, tag="mm", name=f"qk_{h}_{jc}")
                nc.tensor.matmul(
                    ps[:],
                    _r(qT_sb[m * DP:m * DP + D, b, :]),
                    _r(kT_sb[m * DP:m * DP + D, b, jc * 512:(jc + 1) * 512]),
                    start=True, stop=True)
                nc.vector.tensor_add(
                    l_sb[:, jc * 512:(jc + 1) * 512], ps[:],
                    bias_sb[:, h, jc * 512:(jc + 1) * 512])
            nc.vector.tensor_reduce(
                mx[:, 1:2], l_sb[:], op=OP.max, axis=AX.X)
            if CSUB < 2:
                continue
            nm = hpool.tile([128, 1], F32, tag="nm")
            nc.vector.tensor_scalar_mul(nm[:], mx[:, 1:2], -1.0)
            probs = hpool.tile([128, N], pdt, tag="probs")
            ssum = hpool.tile([128, 1], F32, tag="ssum")
            nc.scalar.activation(probs[:], l_sb[:], AF.Exp, bias=nm[:],
                                 accum_out=ssum[:])
            nc.vector.reciprocal(rinv_sb[:, h:h + 1], ssum[:])
            if CSUB < 3:
                continue
            pT = transpose_to(hpool, probs[:], NT, "pT", dt=pdt)
            po = pp_o.tile([128, D], F32, tag="oo", name=f"po_{h}")
            for jb in range(NT):
                nc.tensor.matmul(
                    po[:], _r(pT[:, jb, :]),
                    _r(v_sb[:, jb, h * D:(h + 1) * D]),
                    start=(jb == 0), stop=(jb == NT - 1))
            nc.scalar.activation(o_sb[:, h * D:(h + 1) * D], po[:], AF.Copy,
                                 scale=rinv_sb[:, h:h + 1])
        if CSUB in (1, 2):
            nc.vector.memset(o_sb[:, 0:1], 0.0)
            nc.vector.tensor_scalar(o_sb[:, 0:1], l_sb[:, 0:1], 1e-6, None,
                                    op0=OP.mult, op1=OP.bypass)

        # =========== phase D: output projection ===========
        opool = ctx.enter_context(tc.tile_pool(name="opool", bufs=1))
        wo_sb = opool.tile([128, CC, C], F32R, tag="wo")
        nc.sync.dma_start(wo_sb[:], wo_w.rearrange("(t p) m -> p t m", p=128))
        og = opool.tile([128, HD], F32R, tag="og")
        nc.vector.tensor_mul(og[:], o_sb[:], g_sb[:])
        ogT = transpose_to(opool, og[:], CC, "ogT", dt=F32R)
        out_s = opool.tile([128, C], F32, tag="outs")
        for fc in range(2):
            ps = pp_mm.tile([128, 384], F32, tag="mm")
            for cc in range(CC):
                nc.tensor.matmul(
                    ps[:], _r(ogT[:, cc, :]),
                    _r(wo_sb[:, cc, fc * 384:(fc + 1) * 384]),
                    start=(cc == 0), stop=False)
            nc.tensor.matmul(
                ps[:], _r(ones_sb[:]), _r(bo_sb[:, fc * 384:(fc + 1) * 384]),
                start=False, stop=True)
            nc.scalar.copy(out_s[:, fc * 384:(fc + 1) * 384], ps[:])
        nc.sync.dma_start(out_blk, out_s[:])

    nc.compile()
    return nc


def _host_prep(a, z, mask, ln_a_w, ln_a_b, ln_z_w, ln_z_b, w_z,
               wq, wk, wv, wg, bg, wo, bo):
    f = np.float32
    wq_f = (ln_a_w[:, None] * wq).astype(f) / math.sqrt(D)
    wk_f = (ln_a_w[:, None] * wk).astype(f)
    wq_pad = np.zeros((C, H * DP), f)
    wk_pad = np.zeros((C, H * DP), f)
    for h in range(H):
        wq_pad[:, h * DP:h * DP + D] = wq_f[:, h * D:(h + 1) * D]
        wk_pad[:, h * DP:h * DP + D] = wk_f[:, h * D:(h + 1) * D]
    wq_orig_pad = np.zeros((C, H * DP), f)
    wk_orig_pad = np.zeros((C, H * DP), f)
    for h in range(H):
        wq_orig_pad[:, h * DP:h * DP + D] = wq[:, h * D:(h + 1) * D] / math.sqrt(D)
        wk_orig_pad[:, h * DP:h * DP + D] = wk[:, h * D:(h + 1) * D]
    shared = {
        "a_full": np.ascontiguousarray(a[0]).astype(f),
        "cq_row": (ln_a_b @ wq_orig_pad).reshape(1, H * DP).astype(f),
        "ck_row": (ln_a_b @ wk_orig_pad).reshape(1, H * DP).astype(f),
        "cv_row": (ln_a_b @ wv).reshape(1, HD).astype(f),
        "cg_row": (ln_a_b @ wg + bg).reshape(1, HD).astype(f),
        "wq_pad": wq_pad,
        "wk_pad": wk_pad,
        "wv_w": (ln_a_w[:, None] * wv).astype(f),
        "wg_w": (ln_a_w[:, None] * wg).astype(f),
        "wo_w": wo.astype(f),
        "wprime": (ln_z_w[:, None] * w_z).astype(f),  # cast below
        "maskb_bc": np.broadcast_to(INF * (mask[0] - 1.0), (128, N))
        .astype(f).copy(),
        "bo_row": bo.reshape(1, C).astype(f),
        "ones_col": np.ones((1, 128), f),
        "ones_row": np.ones((1, 512), f),
        "eye_f32": np.eye(128, dtype=f),
        "eye_r": np.eye(128, dtype=f),
        "eye_bf16": np.eye(128, dtype=f),  # cast below
    }
    import ml_dtypes
    shared["eye_bf16"] = shared["eye_bf16"].astype(ml_dtypes.bfloat16)
    shared["wprime"] = shared["wprime"].astype(ml_dtypes.bfloat16)
    return shared


def _run(inputs, trace=False, **spmd_kwargs):
    shared = _host_prep(**inputs)
    a, z = inputs["a"], inputs["z"]
    nc = build_program()
    in_maps = []
    for r in range(NCORES):
        m = dict(shared)
        m["a_own"] = np.ascontiguousarray(
            a[0, r * RB:(r + 1) * RB]).astype(np.float32)
        m["z_blk"] = np.ascontiguousarray(
            z[0, r * RB:(r + 1) * RB]).astype(np.float32)
        in_maps.append(m)
    res = run_bass_kernel_spmd(nc, in_maps, list(range(NCORES)),
                               trace=trace, **spmd_kwargs)
    out = np.concatenate([res.results[r]["out_blk"] for r in range(NCORES)],
                         axis=0)
    return out.reshape(1, N, C).astype(np.float32), res


def kernel(**inputs):
    out, _ = _run(inputs)
    return out


if __name__ == "__main__":
    nc = build_program()
    print("program built ok")



# revision 9
# speedup vs baseline: 117.2162x; 1.7645x over previous
"""AttentionPairBias Trainium2 Bass kernel.

Shapes (hardcoded): B=1, N=1024, C=768, CZ=128, H=16, D=48.
Sharding: query rows i split across 8 cores (128 rows each). Each core
reads its z row-block z[0, 128r:128r+128] (64MB), the full a, and the
weights; it produces output rows [128, 768]. Host gathers row blocks.

Math notes:
 - ln_a_w is folded into wq/wk/wv/wg on host; ln_a_b applied on device.
 - pair bias: w' = ln_z_w * w_z folded on host; bias = inv*(z-mu)@w'.
   The ln_z_b term (ln_z_b @ w_z) is constant over j -> softmax invariant
   -> dropped.
 - q scale 1/sqrt(D) folded into wq on host.
 - mask bias INF*(mask-1) is built on host as a [128,1024] broadcast and
   added to the assembled pair bias once per head slice.
"""

import math
import numpy as np
from contextlib import ExitStack

import concourse.bass as bass
import concourse.bacc as bacc
import concourse.mybir as mybir
import concourse.tile as tile
from concourse.bass_utils import run_bass_kernel_spmd

N, C, CZ, H, D = 1024, 768, 128, 16, 48
HD = H * D
NCORES = 8
RB = N // NCORES          # 128 rows per core
EPS = 1e-5
INF = 1e9
DP = 64                   # padded head dim (2 heads per 128 partitions)

F32 = mybir.dt.float32
F32R = mybir.dt.float32r
BF16 = mybir.dt.bfloat16
AX = mybir.AxisListType
AF = mybir.ActivationFunctionType
OP = mybir.AluOpType

# toggles
import os
PHASES = os.environ.get("KPHASES", "ABC")
KREPEAT = int(os.environ.get("KREPEAT", "1"))
Z_BF16 = True        # bf16 z pipeline after stats/normalize
ZT_CAST = False      # cast z->bf16 during the DMA itself (SWDGE)
PROBS_BF16 = False   # attention probs bf16 for transpose/o-matmul
BIAS_BF16 = True     # assembled pair-bias tensor dtype (SBUF budget needs it)


def _r(ap):
    """fp32 -> fp32r view for full-speed PE streaming."""
    return ap.bitcast(F32R) if ap.dtype == F32 else ap


def build_program():
    nc = bacc.Bacc("TRN2", target_bir_lowering=False, debug=False)

    def din(name, shape, dt=F32):
        return nc.dram_tensor(name, list(shape), dt,
                              kind="ExternalInput").ap()

    a_full = din("a_full", (N, C))
    a_own = din("a_own", (RB, C))
    z_blk = din("z_blk", (RB, N, CZ))
    wq_pad = din("wq_pad", (C, H * DP), F32R)
    wk_pad = din("wk_pad", (C, H * DP), F32R)
    wv_w = din("wv_w", (C, HD), F32R)
    wg_w = din("wg_w", (C, HD), F32R)
    wo_w = din("wo_w", (HD, C), F32R)
    wprime = nc.dram_tensor("wprime", [CZ, H], BF16,
                            kind="ExternalInput").ap()
    cq_row = din("cq_row", (1, H * DP), F32R)
    ck_row = din("ck_row", (1, H * DP), F32R)
    cv_row = din("cv_row", (1, HD), F32R)
    cg_row = din("cg_row", (1, HD), F32R)
    maskb_bc = din("maskb_bc", (128, N))
    bo_row = din("bo_row", (1, C), F32R)
    ones_col = din("ones_col", (1, 128), F32R)
    ones_row = din("ones_row", (1, 512), F32R)
    eye_f32 = din("eye_f32", (128, 128))
    eye_r = din("eye_r", (128, 128), F32R)
    eye_bf16 = nc.dram_tensor("eye_bf16", [128, 128], BF16,
                              kind="ExternalInput").ap()
    out_blk = nc.dram_tensor("out_blk", [RB, C], F32,
                             kind="ExternalOutput").ap()

    zdt = BF16 if Z_BF16 else F32
    pdt = BF16 if PROBS_BF16 else F32R
    bdt = BF16 if BIAS_BF16 else F32

    with tile.TileContext(nc) as tc:
     for _rep in range(KREPEAT):
      sfx = f"_{_rep}" if KREPEAT > 1 else ""
      with ExitStack() as ctx:
        const = ctx.enter_context(tc.tile_pool(name="const" + sfx, bufs=1))
        persist = ctx.enter_context(tc.tile_pool(name="persist", bufs=1))
        pp_mm = ctx.enter_context(tc.tile_pool(name="pp_mm", bufs=3, space="PSUM"))
        pp_t = ctx.enter_context(tc.tile_pool(name="pp_t", bufs=2, space="PSUM"))
        pp_p = ctx.enter_context(tc.tile_pool(name="pp_p", bufs=1, space="PSUM"))
        pp_o = ctx.enter_context(tc.tile_pool(name="pp_o", bufs=1, space="PSUM"))

        # ---- constants to SBUF ----
        ident = const.tile([128, 128], F32)
        nc.sync.dma_start(ident[:], eye_f32)
        ident_b = const.tile([128, 128], BF16)
        nc.sync.dma_start(ident_b[:], eye_bf16)
        ident_r = const.tile([128, 128], F32R)
        nc.sync.dma_start(ident_r[:], eye_r)
        wp_sb = const.tile([CZ, H], BF16)
        nc.sync.dma_start(wp_sb[:], wprime)
        cq_sb = const.tile([1, H * DP], F32R)
        nc.sync.dma_start(cq_sb[:], cq_row)
        ck_sb = const.tile([1, H * DP], F32R)
        nc.sync.dma_start(ck_sb[:], ck_row)
        cv_sb = const.tile([1, HD], F32R)
        nc.sync.dma_start(cv_sb[:], cv_row)
        cg_sb = const.tile([1, HD], F32R)
        nc.sync.dma_start(cg_sb[:], cg_row)
        onesr_sb = const.tile([1, 512], F32R)
        nc.sync.dma_start(onesr_sb[:], ones_row)
        maskb_sb = const.tile([128, N], F32)
        nc.sync.dma_start(maskb_sb[:], maskb_bc)
        bo_sb = const.tile([1, C], F32R)
        nc.sync.dma_start(bo_sb[:], bo_row)
        ones_sb = const.tile([1, 128], F32R)
        nc.sync.dma_start(ones_sb[:], ones_col)
        eps_t = const.tile([128, 1], F32)
        nc.vector.memset(eps_t[:], EPS)

        # persistent activations
        kT_sb = persist.tile([128, 8, N], F32R, tag="kT")       # [2-head blk, b, j]
        v_sb = persist.tile([128, 8, HD], F32R, tag="v")        # [jp, jb, hd]
        qT_sb = persist.tile([128, 8, RB], F32R, tag="qT")      # [2-head blk, b, i]
        g_sb = persist.tile([128, HD], F32, tag="g")           # [i, hd]
        bias_sb = persist.tile([128, H, N], bdt, tag="bias")   # [i, h, j]
        rinv_sb = persist.tile([128, H], F32, tag="rinv")      # softmax 1/sum

        NT = N // 128  # 8 row tiles
        CC = C // 128  # 6 contraction chunks

        def layer_norm_tiles(pool, src_ap, ntiles, name):
            """LN over C for [ntiles*128, C] DRAM input -> SBUF [128, nt, C]."""
            x = pool.tile([128, ntiles, C], F32, tag=f"{name}_x")
            nc.sync.dma_start(
                x[:], src_ap.rearrange("(t p) c -> p t c", p=128))
            xn = pool.tile([128, ntiles, C], F32, tag=f"{name}_n")
            for t in range(ntiles):
                st = pool.tile([128, 2, 6], F32, tag=f"{name}_st")
                for g in range(2):
                    nc.vector.bn_stats(
                        st[:, g, :], x[:, t, g * 384:(g + 1) * 384])
                ag = pool.tile([128, 2], F32, tag=f"{name}_ag")
                nc.vector.bn_aggr(ag[:], st[:])
                sd = pool.tile([128, 1], F32, tag=f"{name}_sd")
                nc.scalar.activation(sd[:], ag[:, 1:2], AF.Sqrt, bias=eps_t[:])
                inv = pool.tile([128, 1], F32, tag=f"{name}_inv")
                nc.vector.reciprocal(inv[:], sd[:])
                nc.vector.tensor_scalar(
                    xn[:, t, :], x[:, t, :], ag[:, 0:1], inv[:],
                    op0=OP.subtract, op1=OP.mult)
            return xn

        def transpose_to(pool, src2d, nblk, name, dt=F32):
            """[128, nblk*128] SBUF -> [128, nblk, 128] transposed SBUF."""
            out = pool.tile([128, nblk, 128], dt, tag=f"{name}_T")
            sdt = src2d.dtype
            idn = {F32: ident[:], BF16: ident_b[:],
                   F32R: ident_r[:]}[sdt]
            b = 0
            while b < nblk:
                take = min(4, nblk - b)
                pt = pp_t.tile([128, 512], sdt, tag="tr")
                for k in range(take):
                    nc.tensor.transpose(
                        pt[:, k * 128:(k + 1) * 128],
                        src2d[:, (b + k) * 128:(b + k + 1) * 128], idn)
                nc.scalar.copy(
                    out[:, b:b + take, :],
                    pt[:, :take * 128].rearrange("p (t x) -> p t x", t=take))
                b += take
            return out

        # =========== phase A: a-side projections ===========
        with tc.tile_pool(name="aph", bufs=1) as aph:

            with tc.tile_pool(name="lnp", bufs=1) as lnp:
                a_n = layer_norm_tiles(lnp, a_full, NT, "af")     # [128, 8, C]
                # a_nT [c-part, cc, n]
                a_nT = aph.tile([128, CC, N], F32R, tag="anT")
                for t in range(NT):
                    pt6 = [pp_t.tile([128, 512], F32, tag="tr", name=f"pt6_{t}_{h2}")
                           for h2 in range(2)]
                    for cc in range(CC):
                        nc.tensor.transpose(
                            pt6[cc // 3][:, (cc % 3) * 128:(cc % 3 + 1) * 128],
                            a_n[:, t, cc * 128:(cc + 1) * 128], ident[:])
                    for half in range(2):
                        nc.scalar.copy(
                            a_nT[:, half * 3:(half + 1) * 3, t * 128:(t + 1) * 128],
                            pt6[half][:, :384].rearrange("p (t x) -> p t x", t=3))

            with tc.tile_pool(name="wkp", bufs=1) as wkp:
                wk_sb = wkp.tile([128, CC, H * DP], F32R, tag="wk")
                nc.sync.dma_start(wk_sb[:],
                                  wk_pad.rearrange("(t p) m -> p t m", p=128))
                # kT_pad [hd-pad block, j]: block b rows = heads 2b,2b+1
                for b in range(NT):
                    for jc in range(2):
                        ps = pp_mm.tile([128, 512], F32, tag="mm", name=f"k_{b}_{jc}")
                        for cc in range(CC):
                            nc.tensor.matmul(
                                ps[:], _r(wk_sb[:, cc, b * 128:(b + 1) * 128]),
                                _r(a_nT[:, cc, jc * 512:(jc + 1) * 512]),
                                start=(cc == 0), stop=(cc == CC - 1))
                        nc.scalar.copy(kT_sb[:, b, jc * 512:(jc + 1) * 512], ps[:])

            with tc.tile_pool(name="wvp", bufs=1) as wvp:
                wv_sb = wvp.tile([128, CC, HD], F32R, tag="wv")
                nc.sync.dma_start(wv_sb[:],
                                  wv_w.rearrange("(t p) m -> p t m", p=128))
                for t in range(NT):
                    for fc in range(2):
                        ps = pp_mm.tile([128, 384], F32, tag="mm", name=f"v_{t}_{fc}")
                        for cc in range(CC):
                            nc.tensor.matmul(
                                ps[:], _r(a_nT[:, cc, t * 128:(t + 1) * 128]),
                                _r(wv_sb[:, cc, fc * 384:(fc + 1) * 384]),
                                start=(cc == 0), stop=(cc == CC - 1))
                        nc.scalar.copy(v_sb[:, t, fc * 384:(fc + 1) * 384], ps[:])

            # own-row LN + q/g
            ao_n = layer_norm_tiles(aph, a_own, 1, "ao")          # [128, 1, C]
            ao_T = transpose_to(aph, ao_n[:, 0, :], CC, "aoT", dt=F32R)    # [c, cc, 128]

            with tc.tile_pool(name="wqp", bufs=1) as wqp:
                wq_sb = wqp.tile([128, CC, H * DP], F32R, tag="wq")
                nc.sync.dma_start(wq_sb[:],
                                  wq_pad.rearrange("(t p) m -> p t m", p=128))
                qtmp = wqp.tile([128, H * DP], F32R, tag="qtmp")
                for fc in range(2):
                    ps = pp_mm.tile([128, 512], F32, tag="mm", name=f"q_{fc}")
                    for cc in range(CC):
                        nc.tensor.matmul(
                            ps[:], _r(ao_T[:, cc, :]),
                            _r(wq_sb[:, cc, fc * 512:(fc + 1) * 512]),
                            start=(cc == 0), stop=(cc == CC - 1))
                    nc.scalar.copy(qtmp[:, fc * 512:(fc + 1) * 512], ps[:])
                qT_t = transpose_to(wqp, qtmp[:], NT, "qT", dt=F32R)
                nc.vector.tensor_copy(qT_sb[:], qT_t[:])

            with tc.tile_pool(name="wgp", bufs=1) as wgp:
                wg_sb = wgp.tile([128, CC, HD], F32R, tag="wg")
                nc.sync.dma_start(wg_sb[:],
                                  wg_w.rearrange("(t p) m -> p t m", p=128))
                # g = sigmoid(ao_n @ wg + bg); bg via rank-1 matmul
                for fc in range(2):
                    ps = pp_mm.tile([128, 384], F32, tag="mm", name=f"g_{fc}")
                    for cc in range(CC):
                        nc.tensor.matmul(
                            ps[:], _r(ao_T[:, cc, :]),
                            _r(wg_sb[:, cc, fc * 384:(fc + 1) * 384]),
                            start=(cc == 0), stop=False)
                    nc.tensor.matmul(
                        ps[:], _r(ones_sb[:]),
                        cg_sb[:, fc * 384:(fc + 1) * 384],
                        start=False, stop=True)
                    nc.scalar.activation(g_sb[:, fc * 384:(fc + 1) * 384], ps[:],
                                         AF.Sigmoid)

        # =========== phase B: z pipeline ===========
        if "B" not in PHASES:
            zrng = []
        else:
            zrng = list(range(RB))
        zpool = ctx.enter_context(tc.tile_pool(name="zpool", bufs=2))
        stpool = ctx.enter_context(tc.tile_pool(name="stage", bufs=1))
        dpool = ctx.enter_context(tc.tile_pool(name="dstage", bufs=2,
                                               space="DRAM"))

        for i in zrng:
            zt = zpool.tile([128, NT, CZ], zdt if ZT_CAST else F32,
                            tag="zt")
            if ZT_CAST:
                nc.gpsimd.dma_start(
                    zt[:], z_blk[i].rearrange("(jb jp) c -> jp jb c", jp=128))
            else:
                nc.sync.dma_start(
                    zt[:], z_blk[i].rearrange("(jb jp) c -> jp jb c", jp=128))
            # stats: one bn_stats via non-mergeable 4D view (group g'=b*2+a
            # holds stats of jb=a*4+b), then per-group bn_aggr.
            st = zpool.tile([128, NT, 8], F32, tag="zst")
            ag = zpool.tile([128, NT, 2], F32, tag="zag")
            for jb in range(NT):
                nc.vector.bn_stats(st[:, jb, 0:6], zt[:, jb, :])
                nc.vector.bn_aggr(ag[:, jb, :], st[:, jb, 0:6])
            sd = zpool.tile([128, NT], F32, tag="zsd")
            nc.scalar.activation(sd[:], ag[:, :, 1], AF.Sqrt, bias=eps_t[:])
            inv = zpool.tile([128, NT], F32, tag="zinv")
            nc.vector.reciprocal(inv[:], sd[:])
            mi = zpool.tile([128, NT], F32, tag="zmi")
            nc.vector.tensor_mul(mi[:], ag[:, :, 0], inv[:])
            nmi = zpool.tile([128, NT], F32, tag="znmi")
            nc.vector.tensor_scalar_mul(nmi[:], mi[:], -1.0)
            zn = zpool.tile([128, NT, CZ], zdt, tag="zn")
            for jb in range(NT):
                nc.gpsimd.tensor_scalar(
                    zn[:, jb, :], zt[:, jb, :], inv[:, jb:jb + 1],
                    nmi[:, jb:jb + 1], op0=OP.mult, op1=OP.add)
            zT = transpose_to(zpool, zn[:].rearrange("p t c -> p (t c)"),
                              NT, "zT", dt=zdt)
            pps = pp_p.tile([16, N], F32, tag="pp", name=f"pps_{i}")
            for half in range(2):
                nc.tensor.matmul(
                    pps[:, half * 512:(half + 1) * 512],
                    _r(wp_sb[:]),
                    _r(zT[:, half * 4:(half + 1) * 4, :]
                       .rearrange("p t x -> p (t x)")),
                    start=True, stop=True)
            if i % 8 == 0:
                pstg = stpool.tile([16, 8, N], bdt, tag="pstg",
                                   name=f"pstg_{i}")
            nc.scalar.copy(pstg[:, i % 8, :], pps[:])
            if i % 8 == 7:
                pb = dpool.tile([16, 8, N], bdt, tag="pb", name=f"pb_{i}")
                # issue on the ACT HWDGE queue: keeps the compute-dependent
                # bias flip off the z-load queue (sync FIFO would serialize
                # the next z DMA behind this tile's whole compute chain)
                nc.scalar.dma_start(pb[:], pstg[:])
                nc.scalar.dma_start(
                    bias_sb[i - 7:i + 1, :, :],
                    pb[:].rearrange("h i j -> i h j"))

        # mask bias add (per h) — bias_sb += maskb
        for h in (range(H) if "B" in PHASES else []):
            nc.vector.tensor_add(bias_sb[:, h, :], bias_sb[:, h, :],
                                 maskb_sb[:])

        # =========== phase C: attention per head ===========
        hpool = ctx.enter_context(tc.tile_pool(name="hpool", bufs=2))
        o_sb = persist.tile([128, HD], F32, tag="o")
        CSUB = 3 if "C" in PHASES else 0
        if "C1" in PHASES:
            CSUB = 1
        elif "C2" in PHASES:
            CSUB = 2
        if CSUB < 3:
            nc.vector.memset(o_sb[:], 1.0)
            nc.vector.memset(rinv_sb[:], 1.0)
        import os as _os
        _hsel = _os.environ.get("KHEADS", "all")
        _heads = (range(H) if _hsel == "all"
                  else range(0, H, 2) if _hsel == "even" else [])
        for h in (_heads if CSUB >= 1 else []):
            b, m = h // 2, h % 2
            l_sb = hpool.tile([128, N], F32, tag="l")
            mx = hpool.tile([128, 2], F32, tag="mx")
            for jc in range(2):
                ps = pp_mm.tile([128, 512], F32, tag="mm", name=f"qk_{h}_{jc}")
                nc.tensor.matmul(
                    ps[:],
                    _r(qT_sb[m * DP:m * DP + D, b, :]),
                    _r(kT_sb[m * DP:m * DP + D, b, jc * 512:(jc + 1) * 512]),
                    start=True, stop=True)
                nc.vector.tensor_add(
                    l_sb[:, jc * 512:(jc + 1) * 512], ps[:],
                    bias_sb[:, h, jc * 512:(jc + 1) * 512])
            nc.vector.tensor_reduce(
                mx[:, 1:2], l_sb[:], op=OP.max, axis=AX.X)
            if CSUB < 2:
                continue
            nm = hpool.tile([128, 1], F32, tag="nm")
            nc.vector.tensor_scalar_mul(nm[:], mx[:, 1:2], -1.0)
            probs = hpool.tile([128, N], pdt, tag="probs")
            ssum = hpool.tile([128, 1], F32, tag="ssum")
            nc.scalar.activation(probs[:], l_sb[:], AF.Exp, bias=nm[:],
                                 accum_out=ssum[:])
            nc.vector.reciprocal(rinv_sb[:, h:h + 1], ssum[:])
            if CSUB < 3:
                continue
            pT = transpose_to(hpool, probs[:], NT, "pT", dt=pdt)
            po = pp_o.tile([128, D], F32, tag="oo", name=f"po_{h}")
            for jb in range(NT):
                nc.tensor.matmul(
                    po[:], _r(pT[:, jb, :]),
                    _r(v_sb[:, jb, h * D:(h + 1) * D]),
                    start=(jb == 0), stop=(jb == NT - 1))
            nc.scalar.activation(o_sb[:, h * D:(h + 1) * D], po[:], AF.Copy,
                                 scale=rinv_sb[:, h:h + 1])
        if CSUB in (1, 2):
            nc.vector.memset(o_sb[:, 0:1], 0.0)
            nc.vector.tensor_scalar(o_sb[:, 0:1], l_sb[:, 0:1], 1e-6, None,
                                    op0=OP.mult, op1=OP.bypass)

        # =========== phase D: output projection ===========
        opool = ctx.enter_context(tc.tile_pool(name="opool", bufs=1))
        wo_sb = opool.tile([128, CC, C], F32R, tag="wo")
        nc.sync.dma_start(wo_sb[:], wo_w.rearrange("(t p) m -> p t m", p=128))
        og = opool.tile([128, HD], F32R, tag="og")
        nc.vector.tensor_mul(og[:], o_sb[:], g_sb[:])
        ogT = transpose_to(opool, og[:], CC, "ogT", dt=F32R)
        out_s = opool.tile([128, C], F32, tag="outs")
        for fc in range(2):
            ps = pp_mm.tile([128, 384], F32, tag="mm")
            for cc in range(CC):
                nc.tensor.matmul(
                    ps[:], _r(ogT[:, cc, :]),
                    _r(wo_sb[:, cc, fc * 384:(fc + 1) * 384]),
                    start=(cc == 0), stop=False)
            nc.tensor.matmul(
                ps[:], _r(ones_sb[:]), _r(bo_sb[:, fc * 384:(fc + 1) * 384]),
                start=False, stop=True)
            nc.scalar.copy(out_s[:, fc * 384:(fc + 1) * 384], ps[:])
        nc.sync.dma_start(out_blk, out_s[:])

    nc.compile()
    return nc


def _host_prep(a, z, mask, ln_a_w, ln_a_b, ln_z_w, ln_z_b, w_z,
               wq, wk, wv, wg, bg, wo, bo):
    f = np.float32
    wq_f = (ln_a_w[:, None] * wq).astype(f) / math.sqrt(D)
    wk_f = (ln_a_w[:, None] * wk).astype(f)
    wq_pad = np.zeros((C, H * DP), f)
    wk_pad = np.zeros((C, H * DP), f)
    for h in range(H):
        wq_pad[:, h * DP:h * DP + D] = wq_f[:, h * D:(h + 1) * D]
        wk_pad[:, h * DP:h * DP + D] = wk_f[:, h * D:(h + 1) * D]
    wq_orig_pad = np.zeros((C, H * DP), f)
    wk_orig_pad = np.zeros((C, H * DP), f)
    for h in range(H):
        wq_orig_pad[:, h * DP:h * DP + D] = wq[:, h * D:(h + 1) * D] / math.sqrt(D)
        wk_orig_pad[:, h * DP:h * DP + D] = wk[:, h * D:(h + 1) * D]
    shared = {
        "a_full": np.ascontiguousarray(a[0]).astype(f),
        "cq_row": (ln_a_b @ wq_orig_pad).reshape(1, H * DP).astype(f),
        "ck_row": (ln_a_b @ wk_orig_pad).reshape(1, H * DP).astype(f),
        "cv_row": (ln_a_b @ wv).reshape(1, HD).astype(f),
        "cg_row": (ln_a_b @ wg + bg).reshape(1, HD).astype(f),
        "wq_pad": wq_pad,
        "wk_pad": wk_pad,
        "wv_w": (ln_a_w[:, None] * wv).astype(f),
        "wg_w": (ln_a_w[:, None] * wg).astype(f),
        "wo_w": wo.astype(f),
        "wprime": (ln_z_w[:, None] * w_z).astype(f),  # cast below
        "maskb_bc": np.broadcast_to(INF * (mask[0] - 1.0), (128, N))
        .astype(f).copy(),
        "bo_row": bo.reshape(1, C).astype(f),
        "ones_col": np.ones((1, 128), f),
        "ones_row": np.ones((1, 512), f),
        "eye_f32": np.eye(128, dtype=f),
        "eye_r": np.eye(128, dtype=f),
        "eye_bf16": np.eye(128, dtype=f),  # cast below
    }
    import ml_dtypes
    shared["eye_bf16"] = shared["eye_bf16"].astype(ml_dtypes.bfloat16)
    shared["wprime"] = shared["wprime"].astype(ml_dtypes.bfloat16)
    return shared


def _run(inputs, trace=False, **spmd_kwargs):
    shared = _host_prep(**inputs)
    a, z = inputs["a"], inputs["z"]
    nc = build_program()
    in_maps = []
    for r in range(NCORES):
        m = dict(shared)
        m["a_own"] = np.ascontiguousarray(
            a[0, r * RB:(r + 1) * RB]).astype(np.float32)
        m["z_blk"] = np.ascontiguousarray(
            z[0, r * RB:(r + 1) * RB]).astype(np.float32)
        in_maps.append(m)
    res = run_bass_kernel_spmd(nc, in_maps, list(range(NCORES)),
                               trace=trace, **spmd_kwargs)
    out = np.concatenate([res.results[r]["out_blk"] for r in range(NCORES)],
                         axis=0)
    return out.reshape(1, N, C).astype(np.float32), res


def kernel(**inputs):
    out, _ = _run(inputs)
    return out


if __name__ == "__main__":
    nc = build_program()
    print("program built ok")

